# revision 120
# baseline (speedup 1.0000x reference)
"""Trainium2 Bass kernel for nn_Attention1 (channel attention transformer block).

Reference computation (per batch):
  kv = W_kv @ x ; k, v = split(kv)                    # pointwise conv over m=3072
  q  = conv3x3(W_q @ y, W_dw)                         # 1x1 then full 3x3, 64x64 image
  q  = linear_interp(snake(q.flatten(HW)), 4096->3072)
  q, k = l2norm over m ; attn = softmax(q @ k^T * temp) per 32-channel head
  out = W_po @ (attn @ v)

Sharding: data-parallel over batch, 16 batches / 8 cores = 2 per core. SPMD,
no collectives; per-core outputs are concatenated on host.

Per-core kernel layout strategy. All heavy matmuls run in fp8(e4m3) with
perf_mode=DoubleRow: operands are laid out [128 partitions, 2, cols] so one
matmul contracts 256 channels (the pair dim sums in the PE), doubling PE
throughput vs fp16. fp8 is safe for everything that feeds the l2-normalized
q/k (scale and elementwise quantization wash out to ~0.1% at score level);
the final W_chain @ x matmul stays fp16 since it writes the output directly.
  - q path   : y kept as fp8 chan-pair images, zero-padded 66x64, three
               horizontally shifted copies; 3x3 conv as 9 DoubleRow matmuls
               (image stationary), both 128-chan blocks contracted at once.
               conv outputs for two row-tiles share one PSUM bank, one copy
               into a contiguous fp8 ct arena (32 x [128,256] slots)
  - snake+interp : fused sparse matrix S applied via one DoubleRow matmul
               per m-tile (S blocks repeat with period 3; the two
               contributing 128x128 blocks ride the pair dim)
  - kT       : x chan-pair fp8 stationary, W_k^T pair fp8 moving -> kT
               (m on partitions), same PSUM bank as interp (one copy)
  - scores   : qk tiles copied to fp8 pair arenas (two m-tiles per pair);
               q@kT via DoubleRow contracting 256 m-rows per matmul
  - norms    : gram blocks qk8[s]^T qk8[s] accumulate in PSUM; diagonal
               extracted with an eye mask (DVE) + ones-matmul -> packed
               [1,512] row of |q|^2,|k|^2 (replaces explicit squares)
  - softmax  : per-head masking via additive -30 mask over the full 256-wide
               score rows; exp on ScalarE with fused row-sum (accum_out);
               1/Z folded into Ahat as a per-row scale
  - out      : W_po @ A @ W_v folded into a 256x256 chain (f32r/f16 small
               matmuls), then W_chain @ x streamed in fp16; output DMA'd
               as f16 and upcast on host
"""
import numpy as np

HEADS = 8
B, DIM, M = 16, 256, 3072
HW = 64
L = HW * HW          # 4096 flattened conv spatial size
NCORES = 8
BL = B // NCORES     # batches per core
C128 = DIM // 128    # channel 128-tiles (2)
NM512 = M // 512     # m-dim 512-tiles (6)
NMT = M // 128       # m-dim 128-tiles (24)
NST = L // 128       # conv-spatial 128-tiles (32)

_CACHE = {}


def _f8(a):
    import ml_dtypes
    return np.asarray(a, np.float32).astype(ml_dtypes.float8_e4m3)


def _f8e5(a):
    import ml_dtypes
    return np.asarray(a, np.float32).astype(ml_dtypes.float8_e5m2)


def _tap_images(y):
    """Fuse conv-shift + snake + linear-interp on the host: for each of the
    9 conv taps, gather the two interp source pixels of each of the m=3072
    output positions from the zero-padded shifted image. The on-chip q path
    is then just 9 DoubleRow matmuls per m-tile against the folded weights
    (computing q directly at 3072 positions instead of 4096 + interp)."""
    mask = np.arange(L).reshape(HW, HW)
    mask[1::2] = mask[1::2][:, ::-1]
    mask = mask.reshape(-1)
    srcp = (np.arange(M) + 0.5) * (L / M) - 0.5
    srcp = np.maximum(srcp, 0.0)
    i0 = np.minimum(np.floor(srcp).astype(np.int64), L - 1)
    i1 = np.minimum(i0 + 1, L - 1)
    lam = (srcp - i0).astype(np.float32)
    g0, g1 = mask[i0], mask[i1]
    y = np.asarray(y, np.float32)
    ypad = np.zeros((B, DIM, HW + 2, HW + 2), np.float32)
    ypad[:, :, 1:HW + 1, 1:HW + 1] = y
    out = np.empty((B, 9, DIM, M), np.float32)
    for dy in range(3):
        for dx in range(3):
            sh = ypad[:, :, dy:dy + HW, dx:dx + HW].reshape(B, DIM, L)
            out[:, dy * 3 + dx] = sh[:, :, g0] * (1 - lam) + sh[:, :, g1] * lam
    return out


def _host_consts(W_kv, W_q, W_dw, W_po, temperature):
    c = np.arange(DIM)
    mask = np.where((c[:, None] // 32) == (c[None, :] // 32), 0.0, -30.0).astype(np.float32)
    tv = np.repeat(np.asarray(temperature, np.float32).reshape(HEADS), DIM // HEADS)
    # folded conv weights: (W_dw . W_q) -> [in-chan b, dy, dx, out o], then
    # chan-pair layout [128 p, 2 t, 9*256] with in-chan = p + 128 t
    fold = np.einsum("oayx,ab->byxo", np.asarray(W_dw, np.float32),
                     np.asarray(W_q, np.float32))
    wdw8 = fold.reshape(2, 128, 9 * DIM).transpose(1, 0, 2)
    # W_k^T chan-pair: [128 p, 2 t, 256 o] with in-chan = p + 128 t
    wk8 = W_kv[:DIM].T.reshape(2, 128, DIM).transpose(1, 0, 2)
    eye4 = np.concatenate([np.eye(128, dtype=np.float32)] * 4, axis=1)
    return {
        "w_k8": _f8(wk8),
        "w_v2": np.ascontiguousarray(W_kv[DIM:], np.float16),
        "w_poT": np.ascontiguousarray(W_po.T, np.float32),
        "w_dw8": _f8(wdw8),
        "eye4": np.ascontiguousarray(eye4),
        "mask": np.ascontiguousarray(mask.reshape(2, 128, DIM)),
        "tempv": np.ascontiguousarray(tv.reshape(2, 128, 1)),
    }


def _make_tc_class():
    """TileContext subclass splitting the end-of-kernel drain waits.

    This container's walrus rejects >1 sem wait on CTRL-encoded instructions
    (Drain/NoOp). The stock Tile epilogue hangs every semaphore's final value
    on one Drain. Emit a chain of SP NoOps with one wait each instead, then a
    waitless drain: SP reaches it only after all sems hit their final values.
    """
    import bass_rust
    import concourse.mybir as mybir
    import concourse.tile as tile

    class SplitDrainTileContext(tile.TileContext):
        def _drain_and_barrier(self, tick_clock, wait_clock):
            probe = self.nc.sync.nop()
            wait_clock.add_sem_waits(
                probe.ins, bass_rust.ScopedClock({None: tick_clock.global_clock})
            )
            waits = list(probe.ins.sync_info.on_wait or [])
            probe.ins.sync_info.on_wait = waits[:1]
            for w in waits[1:]:
                n2 = self.nc.sync.nop()
                n2.ins.sync_info = mybir.SyncInfo(on_wait=[w], on_update=[])
            self.nc.sync.drain()
            self.nc.all_engine_barrier()
            assert self.sems is not None
            popped = self.nc._tile_sem_poison_stack.pop()
            assert popped is self._sem_poison
            self.nc.clear_and_free_semaphores(list(self.sems.allocated().values()))
            self.nc.all_engine_barrier()

    return SplitDrainTileContext


def _split_waits(nc):
    """Walrus in this container allows only one sem wait per instruction.
    Move extra waits onto same-engine NoOps inserted just before."""
    import concourse.mybir as mybir
    n = 0
    for f in nc.m.functions:
        for bb in f.blocks:
            out = []
            changed = False
            for inst in bb.instructions:
                si = inst.sync_info
                waits = list(si.on_wait) if si and si.on_wait else []
                if len(waits) > 1:
                    for w in waits[:-1]:
                        n += 1
                        nop = mybir.InstNoOp(name=f"I-sw{n}-{inst.name}", ins=[], outs=[])
                        nop.engine = inst.engine
                        nop.sync_info = mybir.SyncInfo(on_wait=[w], on_update=[])
                        out.append(nop)
                    si.on_wait = [waits[-1]]
                    changed = True
                out.append(inst)
            if changed:
                bb.instructions = out
    return n


def build_nc(split_waits=True, n_batches=BL, seq=None):
    from contextlib import ExitStack
    from collections import defaultdict
    import concourse.bass as bass
    import concourse.mybir as mybir
    from concourse.masks import make_identity

    f32 = mybir.dt.float32
    f32r = mybir.dt.float32r
    u32 = mybir.dt.uint32
    u8 = mybir.dt.uint8
    f16 = mybir.dt.float16
    f8 = mybir.dt.float8e4
    DR = mybir.MatmulPerfMode.DoubleRow
    Exp = mybir.ActivationFunctionType.Exp
    Ln = mybir.ActivationFunctionType.Ln
    AxX = mybir.AxisListType.X

    def r(ap):
        return ap.bitcast(f32r)

    TC = _make_tc_class()
    nc = bass.Bass("TRN2", target_bir_lowering=False, debug=False)

    f8e5 = mybir.dt.float8e5
    x8d = nc.dram_tensor("x8_sh", [BL, 128, 2, M], f8, kind="ExternalInput").ap()
    x5d = nc.dram_tensor("x5_sh", [BL, 128, 2, M], f8e5, kind="ExternalInput").ap()
    qad = nc.dram_tensor("qa8_sh", [BL, 9, 128, 2, M], f8, kind="ExternalInput").ap()
    wkd = nc.dram_tensor("w_k8", [128, 2, DIM], f8, kind="ExternalInput").ap()
    wvd = nc.dram_tensor("w_v2", [DIM, DIM], f16, kind="ExternalInput").ap()
    wpd = nc.dram_tensor("w_poT", [DIM, DIM], f32r, kind="ExternalInput").ap()
    wdd = nc.dram_tensor("w_dw8", [128, 2, 9 * DIM], f8, kind="ExternalInput").ap()
    eyd = nc.dram_tensor("eye4", [128, 512], f32, kind="ExternalInput").ap()
    md = nc.dram_tensor("mask", [2, 128, DIM], f32, kind="ExternalInput").ap()
    td = nc.dram_tensor("tempv", [2, 128, 1], f32, kind="ExternalInput").ap()
    od = nc.dram_tensor("out", [BL, DIM, M], f16, kind="ExternalOutput").ap()

    with TC(nc) as tc, ExitStack() as ctx:
        P = lambda **kw: ctx.enter_context(tc.tile_pool(**kw))
        consts = P(name="consts", bufs=1)
        p_qp = P(name="p_qp", bufs=2)
        p_x = P(name="p_x", bufs=2)
        p_qk = P(name="p_qk", bufs=4)
        p_sm = P(name="p_sm", bufs=2)
        p_tn = P(name="p_tn", bufs=4)
        p_fin = P(name="p_fin", bufs=6)
        # global PSUM pools: 4+1+1+2 = 8 banks exactly
        pp_ik = P(name="pp_ik", bufs=4, space="PSUM")
        pp_sc = P(name="pp_sc", bufs=1, space="PSUM")
        pp_g4 = P(name="pp_g4", bufs=1, space="PSUM")
        pp_pq = P(name="pp_pq", bufs=2, space="PSUM")

        # ---- constants; the conv weights are split across all three DMA
        # queues (behind each queue's first image chunk) so the first conv
        # tile can start ~2.5us in; softmax/out-chain consts are emitted
        # after batch 0's loads ----
        wdw8 = consts.tile([128, 2, 9 * DIM], f8, tag="wdw8", name="wdw8")
        wk8 = consts.tile([128, 2, DIM], f8, tag="wk8", name="wk8")

        def early_consts():
            for dy, eng in ((0, nc.scalar), (1, nc.sync), (2, nc.gpsimd)):
                c0, c1 = 3 * DIM * dy, 3 * DIM * (dy + 1)
                eng.dma_start(out=wdw8[:, :, c0:c1], in_=wdd[:, :, c0:c1])
            nc.scalar.dma_start(out=wk8[:], in_=wkd)
            # dummy activation primes the copy/exp/ln table while the image
            # DMAs land, so the first real ACT copy doesn't pay the ~1.3us
            # table load mid-stream
            nc.scalar.copy(scr[:], ones_row[:])
        wv2 = [consts.tile([128, DIM], f16, tag=f"wv2{k}", name=f"wv2{k}") for k in range(C128)]
        wp = [consts.tile([128, DIM], f32r, tag=f"wp{k}", name=f"wp{k}") for k in range(C128)]
        eye4 = consts.tile([128, 512], f32, tag="eye4", name="eye4")
        msk = [consts.tile([128, DIM], f32, tag=f"msk{k}", name=f"msk{k}") for k in range(2)]
        tmpv = [consts.tile([128, 1], f32, tag=f"tmpv{k}", name=f"tmpv{k}") for k in range(2)]

        def late_consts():
            for k in range(C128):
                sl = slice(128 * k, 128 * (k + 1))
                nc.sync.dma_start(out=wv2[k][:], in_=wvd[sl, :])
                nc.sync.dma_start(out=wp[k][:], in_=wpd[sl, :])
            nc.sync.dma_start(out=eye4[:], in_=eyd)
            for rr in range(2):
                nc.sync.dma_start(out=msk[rr][:], in_=md[rr])
                nc.sync.dma_start(out=tmpv[rr][:], in_=td[rr])
        ident = consts.tile([128, 128], f32, tag="ident", name="ident")
        make_identity(nc, ident[:])
        ones_row = consts.tile([1, 128], f32r, tag="ones", name="ones")
        onesf = consts.tile([1, 128], f32, tag="onesf", name="onesf")
        nc.vector.memset(onesf[:], 1.0)
        nc.vector.tensor_copy(ones_row[:], onesf[:])
        scr = consts.tile([1, 128], f32, tag="scr", name="scr")
        ones_c16 = consts.tile([128, 1], f16, tag="ones16", name="ones16")
        nc.vector.memset(ones_c16[:], 1.0)

        state = defaultdict(dict)

        def emit_load_q1(vk, b, after_c0=None):
            s = state[vk]
            s["b"] = b
            s["x8"] = p_x.tile([128, 2, M], f8, tag="x8", name="x8")
            # the q path input is 9 host-precomputed tap images, already
            # snake+interp-transformed to the m=3072 output positions; each
            # tap is two m-half tiles (whole-tile deps gate per tile).
            # batch 0 spreads firsts over all three queues for startup; later
            # batches avoid the ACT queue (it carries the PSUM copies)
            qa = [[p_qp.tile([128, 2, M // 2], f8, tag=f"qa{t}{h}",
                             name=f"qa{t}{h}") for h in range(2)]
                  for t in range(9)]
            s["qa"] = qa
            engs = (nc.sync, nc.scalar, nc.gpsimd)
            ne = len(engs)
            for h in range(2):
                for t in range(9):
                    engs[(h * 9 + t) % ne].dma_start(
                        out=qa[t][h][:],
                        in_=qad[b, t][:, :, 1536 * h:1536 * (h + 1)])
                if h == 0 and after_c0 is not None:
                    after_c0()
            for cc in range(2):
                nc.sync.dma_start(
                    out=s["x8"][:, :, 1536 * cc:1536 * (cc + 1)],
                    in_=x8d[b][:, :, 1536 * cc:1536 * (cc + 1)])
            # e5m2 residual of x (for the fp8 error-feedback output matmul)
            s["x5"] = p_x.tile([128, 2, M], f8e5, tag="x5", name="x5")
            nc.gpsimd.dma_start(out=s["x5"][:], in_=x5d[b])

        def emit_stream(vk, hooks=()):
            s = state[vk]
            qa, x8t = s["qa"], s["x8"]
            ps_scc = pp_sc.tile([128, 512], f32, tag="pscc", name="pscc")
            ps_sc = [ps_scc[:, 0:DIM], ps_scc[:, DIM:512]]
            ps_g4 = pp_g4.tile([128, 512], f32, tag="g4", name="g4")
            s["sc"], s["scc"], s["g4"] = ps_sc, ps_scc, ps_g4
            qps = s["qps"] = []

            def emit_mtile(j):
                h, off = divmod(128 * j, M // 2)
                psik = pp_ik.tile([128, 512], f32, tag="pik", name="pik")
                # q = sum over 9 taps of (tap image @ folded weights), direct
                # at the 3072 interp positions; kT into [256:512] of the bank
                for t in range(9):
                    nc.tensor.matmul(
                        psik[:, 0:DIM], qa[t][h][:, :, off:off + 128],
                        wdw8[:, :, t * DIM:(t + 1) * DIM],
                        start=(t == 0), stop=False,
                        perf_mode=DR, skip_group_check=True)
                nc.tensor.matmul(
                    psik[:, DIM:512], x8t[:, :, 128 * j:128 * (j + 1)], wk8[:],
                    start=False, stop=True, perf_mode=DR, skip_group_check=True)
                if j % 2 == 0:
                    qps.append(p_qk.tile([128, 2, 512], f8, tag="qk8", name="qk8"))
                    nc.vector.tensor_copy(qps[-1][:, 0, :], psik[:])
                else:
                    nc.scalar.copy(qps[-1][:, 1, :], psik[:])

            def emit_scores(p):
                # lagged a few mtiles so the qk8 copies are long done
                qp = qps[p]
                for rr in range(2):
                    nc.tensor.matmul(
                        ps_sc[rr], qp[:, :, 128 * rr:128 * (rr + 1)],
                        qp[:, :, DIM:512],
                        start=(p == 0 and rr == 0),
                        stop=(p == NMT // 2 - 1 and rr == 1),
                        perf_mode=DR, skip_group_check=True)
                # gram blocks for l2 norms (diag extracted in softmax)
                for g in range(4):
                    nc.tensor.matmul(
                        ps_g4[:, 128 * g:128 * (g + 1)],
                        qp[:, :, 128 * g:128 * (g + 1)],
                        qp[:, :, 128 * g:128 * (g + 1)],
                        start=(p == 0), stop=(p == NMT // 2 - 1),
                        perf_mode=DR, skip_group_check=True)

            ndone = 0
            for j in range(NMT):
                emit_mtile(j)
                if j % 3 == 2 and j // 3 < len(hooks):
                    hooks[j // 3]()
                ready = max(0, (j - 3) // 2)
                while ndone < ready:
                    emit_scores(ndone)
                    ndone += 1

            def tail_scores(nd=ndone):
                for p in range(nd, NMT // 2):
                    emit_scores(p)

            # defer the last score pairs: their qk8 copies were just issued,
            # so the caller schedules them under later PE work
            s["tail_fn"] = tail_scores

        def softmax_parts(vk):
            """Softmax chain as three emission hooks, interleavable with the
            next batch's conv groups so the chain latency hides under them.
            Part 1 frees the pscc/g4 PSUM banks early (SBUF copies)."""
            s = state[vk]
            rqT, rZ = [], []

            def part1():
                s["tail_fn"]()  # this batch's deferred score pairs
                ps_g4 = s["g4"]
                # scores to SBUF, freeing the bank for the next stream
                scp = p_sm.tile([128, 512], f32, tag="scp", name="scp")
                s["scp"] = scp
                nc.scalar.copy(scp[:], s["scc"][:])
                # gram diagonals via eye mask
                geye = p_sm.tile([128, 512], f16, tag="geye", name="geye")
                nc.vector.tensor_mul(geye[:], ps_g4[:], eye4[:])
                # q norms straight to per-partition columns via DVE reduce
                # (skips the ones-matmul + PE transposes on the q side);
                # 1/sqrt as exp(-ln/2) keeps all ACT funcs in one table
                nq2 = p_tn.tile([128, 2], f32, tag="nq2", name="nq2")
                nc.vector.tensor_reduce(nq2[:, 0:1], geye[:, 0:128],
                                        axis=AxX, op=mybir.AluOpType.add)
                nc.vector.tensor_reduce(nq2[:, 1:2], geye[:, 128:256],
                                        axis=AxX, op=mybir.AluOpType.add)
                rq2 = p_tn.tile([128, 2], f32, tag="rq2", name="rq2")
                nc.scalar.activation(rq2[:], nq2[:], Ln)
                nc.scalar.activation(rq2[:], rq2[:], Exp, scale=-0.5)
                for rr in range(2):
                    rqt = p_tn.tile([128, 1], f32, tag="rqt", name="rqt")
                    nc.vector.tensor_mul(rqt[:], rq2[:, rr:rr + 1], tmpv[rr][:])
                    rqT.append(rqt)
                # k norms as a row (for the partition-broadcast matmul)
                ps_nqk = pp_pq.tile([1, DIM], f32, tag="pq", name="pq")
                nc.tensor.matmul(ps_nqk[:], ones_c16[:], geye[:, DIM:512],
                                 start=True, stop=True)
                rrow = p_sm.tile([1, DIM], f32, tag="rrow", name="rrow", bufs=2)
                nc.scalar.activation(rrow[:], ps_nqk[:], Ln)
                nc.scalar.activation(rrow[:], rrow[:], Exp, scale=-0.5)
                # DVE copy rounds to f32r so the broadcast matmul runs 4x
                # faster than plain f32
                rrow_r = p_sm.tile([1, DIM], f32r, tag="rrowr", name="rrowr",
                                   bufs=2)
                s["rrow"] = rrow_r
                nc.vector.tensor_copy(rrow_r[:], rrow[:])

            def part2():
                rrow = s["rrow"]
                # rnk broadcast down partitions via outer product; the sc
                # chain reads it straight from PSUM (no staging copy)
                psb = pp_pq.tile([128, DIM], f32, tag="pq", name="pq")
                s["psb"] = psb
                nc.tensor.matmul(psb[:], ones_row[:], rrow[:],
                                 start=True, stop=True)

            def part3():
                scp, rkb = s["scp"], s["psb"]
                E = [p_sm.tile([128, DIM], f32, tag="e", name="e") for _ in range(2)]
                # masked softmax, exp with fused row-sum; the all-SBUF chain
                # ops legally run on GPSIMD/Pool, freeing DVE for PSUM copies
                for rr in range(2):
                    sc = p_sm.tile([128, DIM], f32, tag="sc", name="sc")
                    # fused (scp * rq) * rkb in one DVE pass
                    nc.vector.scalar_tensor_tensor(
                        sc[:], scp[:, DIM * rr:DIM * (rr + 1)], rqT[rr][:],
                        rkb[:], op0=mybir.AluOpType.mult,
                        op1=mybir.AluOpType.mult)
                    nc.gpsimd.tensor_add(sc[:], sc[:], msk[rr][:])
                    z = p_tn.tile([128, 1], f32, tag="z", name="z")
                    nc.scalar.activation(E[rr][:], sc[:], Exp, accum_out=z[:])
                    rz = p_tn.tile([128, 1], f32, tag="rz", name="rz")
                    nc.vector.reciprocal(rz[:], z[:])
                    rZ.append(rz)
                # Ahat = E / Z (rows)
                Ahat = [p_sm.tile([128, DIM], f32r, tag="ah", name="ah", bufs=3)
                        for _ in range(2)]
                for rr in range(2):
                    nc.vector.tensor_scalar_mul(Ahat[rr][:], E[rr][:], rZ[rr][:])
                s["Ahat"] = Ahat

            return (part1, part2, part3)

        def emit_softmax(vk):
            for p in softmax_parts(vk):
                p()

        def out_head(vk):
            s = state[vk]
            Ahat = s["Ahat"]
            # m1t[d,o] = (W_po @ Ahat)^T ; wch[c,o] = (W_po @ Ahat @ W_v)^T
            m1t = [p_sm.tile([128, DIM], f16, tag="m1t", name="m1t") for _ in range(2)]
            for d in range(2):
                ps = pp_pq.tile([128, DIM], f32, tag="pq", name="pq")
                for k in range(C128):
                    nc.tensor.matmul(
                        ps[:], r(Ahat[k][:, 128 * d:128 * (d + 1)]), r(wp[k][:]),
                        start=(k == 0), stop=(k == C128 - 1))
                nc.scalar.copy(m1t[d][:], ps[:])
            # wch in e4m3 + e5m2 residual: the final matmul runs as fp8
            # DoubleRow with error feedback (wch8@x8 + wch8@dx + dwch@x8),
            # adding only ~0.2% output error but halving the PE cost
            wch8 = p_sm.tile([128, 2, DIM], f8, tag="wch8", name="wch8")
            dwch5 = p_sm.tile([128, 2, DIM], f8e5, tag="dwch5", name="dwch5")
            s["wch8"], s["dwch5"] = wch8, dwch5
            for cb in range(2):
                ps = pp_pq.tile([128, DIM], f32, tag="pq", name="pq")
                for d in range(2):
                    nc.tensor.matmul(
                        ps[:], wv2[d][:, 128 * cb:128 * (cb + 1)], m1t[d][:],
                        start=(d == 0), stop=(d == 1))
                nc.scalar.copy(wch8[:, cb, :], ps[:])
                nc.vector.tensor_sub(dwch5[:, cb, :], ps[:], wch8[:, cb, :])

        def fin_tile(vk, i, mixed=True):
            """One 1024-wide output tile of W_chain @ x (fp8 DoubleRow with
            error feedback). mixed=False keeps PSUM in the pq pool so tiles
            can interleave with a stream (whose psik owns the ik pool)."""
            s = state[vk]
            b = s["b"]
            x8t, x5t = s["x8"], s["x5"]
            wch8, dwch5 = s["wch8"], s["dwch5"]
            o, h = divmod(i, NM512 // 2)
            st = p_fin.tile([128, 1024], f16, tag="fin", name="fin")
            for half in range(2):
                n = 2 * h + half
                pool = pp_pq if (not mixed or (2 * i + half) % 2 == 0) else pp_ik
                tg = "pq" if pool is pp_pq else "pik"
                ps = pool.tile([128, 512], f32, tag=tg, name=tg)
                osl = slice(128 * o, 128 * (o + 1))
                nsl = slice(512 * n, 512 * (n + 1))
                nc.tensor.matmul(ps[:], wch8[:, :, osl], x8t[:, :, nsl],
                                 start=True, stop=False, perf_mode=DR)
                nc.tensor.matmul(ps[:], wch8[:, :, osl], x5t[:, :, nsl],
                                 start=False, stop=False, perf_mode=DR)
                nc.tensor.matmul(ps[:], dwch5[:, :, osl], x8t[:, :, nsl],
                                 start=False, stop=True, perf_mode=DR)
                if half == 0:
                    nc.scalar.copy(st[:, 0:512], ps[:])
                else:
                    nc.vector.tensor_copy(st[:, 512:1024], ps[:])
            deng = nc.sync if h % 2 == 0 else nc.gpsimd
            deng.dma_start(
                out=od[b, 128 * o:128 * (o + 1), 1024 * h:1024 * (h + 1)],
                in_=st[:])

        def fin_tile_split(vk, i):
            # last tile of the kernel: two half staging tiles with parallel
            # DMAs on separate queues shortens the post-matmul drain chain
            s = state[vk]
            b = s["b"]
            x8t, x5t = s["x8"], s["x5"]
            wch8, dwch5 = s["wch8"], s["dwch5"]
            o, h = divmod(i, NM512 // 2)
            for half in range(2):
                n = 2 * h + half
                pool = pp_pq if half == 0 else pp_ik
                tg = "pq" if pool is pp_pq else "pik"
                ps = pool.tile([128, 512], f32, tag=tg, name=tg)
                osl = slice(128 * o, 128 * (o + 1))
                nsl = slice(512 * n, 512 * (n + 1))
                nc.tensor.matmul(ps[:], wch8[:, :, osl], x8t[:, :, nsl],
                                 start=True, stop=False, perf_mode=DR)
                nc.tensor.matmul(ps[:], wch8[:, :, osl], x5t[:, :, nsl],
                                 start=False, stop=False, perf_mode=DR)
                nc.tensor.matmul(ps[:], dwch5[:, :, osl], x8t[:, :, nsl],
                                 start=False, stop=True, perf_mode=DR)
                st = p_fin.tile([128, 512], f16, tag="fin2", name="fin2")
                if half == 0:
                    nc.scalar.copy(st[:], ps[:])
                else:
                    nc.vector.tensor_copy(st[:], ps[:])
                deng = nc.sync if half == 0 else nc.gpsimd
                deng.dma_start(
                    out=od[b, 128 * o:128 * (o + 1), 512 * n:512 * (n + 1)],
                    in_=st[:])

        def emit_out(vk, hooks=(), tiles=None, split_last=False):
            s = state[vk]
            if "wch8" not in s:
                out_head(vk)
            if len(hooks) > 0:
                hooks[0]()
            idxs = list(range(2 * (NM512 // 2)) if tiles is None else tiles)
            for i in idxs:
                if split_last and i == idxs[-1]:
                    fin_tile_split(vk, i)
                else:
                    fin_tile(vk, i)
                if i + 1 < len(hooks):
                    hooks[i + 1]()

        # software pipeline: q1(b+1) fills the PE while batch b's softmax
        # chain runs *inside* stream(b+1) via hooks (its tiny PE ops slot
        # between conv groups; part 1 frees the score/gram PSUM banks before
        # stream(b+1)'s first scores matmul needs them)
        sq_ = list(range(n_batches)) if seq is None else list(seq)
        vis = [(i, b) for i, b in enumerate(sq_)]
        n = len(vis)
        emit_load_q1(0, vis[0][1], after_c0=early_consts)
        emit_stream(0)
        late_consts()
        for i in range(1, n):
            emit_load_q1(i, vis[i][1])
            # softmax(i-1) and batch i-1's out-head hide inside stream(i)'s
            # conv groups
            sm = softmax_parts(i - 1)
            emit_stream(i, hooks=(*sm, lambda vv=i - 1: out_head(vv)))
            if i >= 2:
                emit_out(i - 2)
        if n > 1:
            # tail: last softmax chain hides under out(n-2)'s fin stream
            emit_out(n - 2, hooks=softmax_parts(n - 1))
            out_head(n - 1)
            emit_out(n - 1, split_last=True)
        else:
            emit_softmax(0)
            emit_out(0)

    if split_waits:
        _split_waits(nc)
    return nc


def _get_nc():
    if "nc" not in _CACHE:
        _CACHE["nc"] = build_nc()
    return _CACHE["nc"]


def make_inputs(inputs):
    """Host-side prep: consts + per-core sharded activations."""
    consts = _host_consts(inputs["W_kv"], inputs["W_q"], inputs["W_dw"],
                          inputs["W_po"], inputs["temperature"])
    xf = np.asarray(inputs["x"], np.float32)
    x8f = _f8(xf)
    x5f = _f8e5(xf - np.float32(x8f))
    x8 = x8f.reshape(B, 2, 128, M).transpose(0, 2, 1, 3)
    x5 = x5f.reshape(B, 2, 128, M).transpose(0, 2, 1, 3)
    qa8 = _f8(_tap_images(inputs["y"])).reshape(B, 9, 2, 128, M)
    qa8 = qa8.transpose(0, 1, 3, 2, 4)
    in_maps = []
    for i in range(NCORES):
        m = dict(consts)
        m["x8_sh"] = np.ascontiguousarray(x8[BL * i:BL * (i + 1)])
        m["x5_sh"] = np.ascontiguousarray(x5[BL * i:BL * (i + 1)])
        m["qa8_sh"] = np.ascontiguousarray(qa8[BL * i:BL * (i + 1)])
        in_maps.append(m)
    return in_maps


def run(inputs, trace=False, trace_kwargs=None):
    from concourse.bass_utils import run_bass_kernel_spmd

    nc = _get_nc()
    in_maps = make_inputs(inputs)
    res = run_bass_kernel_spmd(
        nc, in_maps, core_ids=list(range(NCORES)), trace=trace,
        trace_kwargs=trace_kwargs or {})
    out = np.concatenate(
        [np.asarray(res.results[i]["out"], np.float32) for i in range(NCORES)],
        axis=0)
    return out, res


def kernel(**inputs) -> np.ndarray:
    out, _ = run(inputs, trace=False)
    return out


# revision 121
# speedup vs baseline: 1.0287x; 1.0287x over previous
"""Trainium2 Bass kernel for nn_Attention1 (channel attention transformer block).

Reference computation (per batch):
  kv = W_kv @ x ; k, v = split(kv)                    # pointwise conv over m=3072
  q  = conv3x3(W_q @ y, W_dw)                         # 1x1 then full 3x3, 64x64 image
  q  = linear_interp(snake(q.flatten(HW)), 4096->3072)
  q, k = l2norm over m ; attn = softmax(q @ k^T * temp) per 32-channel head
  out = W_po @ (attn @ v)

Sharding: data-parallel over batch, 16 batches / 8 cores = 2 per core. SPMD,
no collectives; per-core outputs are concatenated on host.

Per-core kernel layout strategy. All heavy matmuls run in fp8(e4m3) with
perf_mode=DoubleRow: operands are laid out [128 partitions, 2, cols] so one
matmul contracts 256 channels (the pair dim sums in the PE), doubling PE
throughput vs fp16. fp8 is safe for everything that feeds the l2-normalized
q/k (scale and elementwise quantization wash out to ~0.1% at score level);
the final W_chain @ x matmul stays fp16 since it writes the output directly.
  - q path   : y kept as fp8 chan-pair images, zero-padded 66x64, three
               horizontally shifted copies; 3x3 conv as 9 DoubleRow matmuls
               (image stationary), both 128-chan blocks contracted at once.
               conv outputs for two row-tiles share one PSUM bank, one copy
               into a contiguous fp8 ct arena (32 x [128,256] slots)
  - snake+interp : fused sparse matrix S applied via one DoubleRow matmul
               per m-tile (S blocks repeat with period 3; the two
               contributing 128x128 blocks ride the pair dim)
  - kT       : x chan-pair fp8 stationary, W_k^T pair fp8 moving -> kT
               (m on partitions), same PSUM bank as interp (one copy)
  - scores   : qk tiles copied to fp8 pair arenas (two m-tiles per pair);
               q@kT via DoubleRow contracting 256 m-rows per matmul
  - norms    : gram blocks qk8[s]^T qk8[s] accumulate in PSUM; diagonal
               extracted with an eye mask (DVE) + ones-matmul -> packed
               [1,512] row of |q|^2,|k|^2 (replaces explicit squares)
  - softmax  : per-head masking via additive -30 mask over the full 256-wide
               score rows; exp on ScalarE with fused row-sum (accum_out);
               1/Z folded into Ahat as a per-row scale
  - out      : W_po @ A @ W_v folded into a 256x256 chain (f32r/f16 small
               matmuls), then W_chain @ x streamed in fp16; output DMA'd
               as f16 and upcast on host
"""
import numpy as np

HEADS = 8
B, DIM, M = 16, 256, 3072
HW = 64
L = HW * HW          # 4096 flattened conv spatial size
NCORES = 8
BL = B // NCORES     # batches per core
C128 = DIM // 128    # channel 128-tiles (2)
NM512 = M // 512     # m-dim 512-tiles (6)
NMT = M // 128       # m-dim 128-tiles (24)
NST = L // 128       # conv-spatial 128-tiles (32)

_CACHE = {}


def _f8(a):
    import ml_dtypes
    return np.asarray(a, np.float32).astype(ml_dtypes.float8_e4m3)


def _f8e5(a):
    import ml_dtypes
    return np.asarray(a, np.float32).astype(ml_dtypes.float8_e5m2)


def _tap_images(y):
    """Fuse conv-shift + snake + linear-interp on the host: for each of the
    9 conv taps, gather the two interp source pixels of each of the m=3072
    output positions from the zero-padded shifted image. The on-chip q path
    is then just 9 DoubleRow matmuls per m-tile against the folded weights
    (computing q directly at 3072 positions instead of 4096 + interp)."""
    mask = np.arange(L).reshape(HW, HW)
    mask[1::2] = mask[1::2][:, ::-1]
    mask = mask.reshape(-1)
    srcp = (np.arange(M) + 0.5) * (L / M) - 0.5
    srcp = np.maximum(srcp, 0.0)
    i0 = np.minimum(np.floor(srcp).astype(np.int64), L - 1)
    i1 = np.minimum(i0 + 1, L - 1)
    lam = (srcp - i0).astype(np.float32)
    g0, g1 = mask[i0], mask[i1]
    y = np.asarray(y, np.float32)
    ypad = np.zeros((B, DIM, HW + 2, HW + 2), np.float32)
    ypad[:, :, 1:HW + 1, 1:HW + 1] = y
    out = np.empty((B, 9, DIM, M), np.float32)
    for dy in range(3):
        for dx in range(3):
            sh = ypad[:, :, dy:dy + HW, dx:dx + HW].reshape(B, DIM, L)
            out[:, dy * 3 + dx] = sh[:, :, g0] * (1 - lam) + sh[:, :, g1] * lam
    return out


def _host_consts(W_kv, W_q, W_dw, W_po, temperature):
    c = np.arange(DIM)
    mask = np.where((c[:, None] // 32) == (c[None, :] // 32), 0.0, -30.0).astype(np.float32)
    tv = np.repeat(np.asarray(temperature, np.float32).reshape(HEADS), DIM // HEADS)
    # folded conv weights: (W_dw . W_q) -> [in-chan b, dy, dx, out o], then
    # chan-pair layout [128 p, 2 t, 9*256] with in-chan = p + 128 t
    fold = np.einsum("oayx,ab->byxo", np.asarray(W_dw, np.float32),
                     np.asarray(W_q, np.float32))
    wdw8 = fold.reshape(2, 128, 9 * DIM).transpose(1, 0, 2)
    # W_k^T chan-pair: [128 p, 2 t, 256 o] with in-chan = p + 128 t
    wk8 = W_kv[:DIM].T.reshape(2, 128, DIM).transpose(1, 0, 2)
    eye4 = np.concatenate([np.eye(128, dtype=np.float32)] * 4, axis=1)
    return {
        "w_k8": _f8(wk8),
        "w_v2": np.ascontiguousarray(W_kv[DIM:], np.float16),
        "w_poT": np.ascontiguousarray(W_po.T, np.float32),
        "w_dw8": _f8(wdw8),
        "eye4": np.ascontiguousarray(eye4),
        "mask": np.ascontiguousarray(mask.reshape(2, 128, DIM)),
        "tempv": np.ascontiguousarray(tv.reshape(2, 128, 1)),
    }


def _make_tc_class():
    """TileContext subclass splitting the end-of-kernel drain waits.

    This container's walrus rejects >1 sem wait on CTRL-encoded instructions
    (Drain/NoOp). The stock Tile epilogue hangs every semaphore's final value
    on one Drain. Emit a chain of SP NoOps with one wait each instead, then a
    waitless drain: SP reaches it only after all sems hit their final values.
    """
    import bass_rust
    import concourse.mybir as mybir
    import concourse.tile as tile

    class SplitDrainTileContext(tile.TileContext):
        def _drain_and_barrier(self, tick_clock, wait_clock):
            probe = self.nc.sync.nop()
            wait_clock.add_sem_waits(
                probe.ins, bass_rust.ScopedClock({None: tick_clock.global_clock})
            )
            waits = list(probe.ins.sync_info.on_wait or [])
            probe.ins.sync_info.on_wait = waits[:1]
            for w in waits[1:]:
                n2 = self.nc.sync.nop()
                n2.ins.sync_info = mybir.SyncInfo(on_wait=[w], on_update=[])
            self.nc.sync.drain()
            self.nc.all_engine_barrier()
            assert self.sems is not None
            popped = self.nc._tile_sem_poison_stack.pop()
            assert popped is self._sem_poison
            self.nc.clear_and_free_semaphores(list(self.sems.allocated().values()))
            self.nc.all_engine_barrier()

    return SplitDrainTileContext


def _split_waits(nc):
    """Walrus in this container allows only one sem wait per instruction.
    Move extra waits onto same-engine NoOps inserted just before."""
    import concourse.mybir as mybir
    n = 0
    for f in nc.m.functions:
        for bb in f.blocks:
            out = []
            changed = False
            for inst in bb.instructions:
                si = inst.sync_info
                waits = list(si.on_wait) if si and si.on_wait else []
                if len(waits) > 1:
                    for w in waits[:-1]:
                        n += 1
                        nop = mybir.InstNoOp(name=f"I-sw{n}-{inst.name}", ins=[], outs=[])
                        nop.engine = inst.engine
                        nop.sync_info = mybir.SyncInfo(on_wait=[w], on_update=[])
                        out.append(nop)
                    si.on_wait = [waits[-1]]
                    changed = True
                out.append(inst)
            if changed:
                bb.instructions = out
    return n


def build_nc(split_waits=True, n_batches=BL, seq=None):
    from contextlib import ExitStack
    from collections import defaultdict
    import concourse.bass as bass
    import concourse.mybir as mybir
    from concourse.masks import make_identity

    f32 = mybir.dt.float32
    f32r = mybir.dt.float32r
    u32 = mybir.dt.uint32
    u8 = mybir.dt.uint8
    f16 = mybir.dt.float16
    f8 = mybir.dt.float8e4
    DR = mybir.MatmulPerfMode.DoubleRow
    Exp = mybir.ActivationFunctionType.Exp
    Ln = mybir.ActivationFunctionType.Ln
    AxX = mybir.AxisListType.X

    def r(ap):
        return ap.bitcast(f32r)

    TC = _make_tc_class()
    nc = bass.Bass("TRN2", target_bir_lowering=False, debug=False)

    f8e5 = mybir.dt.float8e5
    x8d = nc.dram_tensor("x8_sh", [BL, 128, 2, M], f8, kind="ExternalInput").ap()
    x5d = nc.dram_tensor("x5_sh", [BL, 128, 2, M], f8e5, kind="ExternalInput").ap()
    qad = nc.dram_tensor("qa8_sh", [BL, 9, 128, 2, M], f8, kind="ExternalInput").ap()
    wkd = nc.dram_tensor("w_k8", [128, 2, DIM], f8, kind="ExternalInput").ap()
    wvd = nc.dram_tensor("w_v2", [DIM, DIM], f16, kind="ExternalInput").ap()
    wpd = nc.dram_tensor("w_poT", [DIM, DIM], f32r, kind="ExternalInput").ap()
    wdd = nc.dram_tensor("w_dw8", [128, 2, 9 * DIM], f8, kind="ExternalInput").ap()
    eyd = nc.dram_tensor("eye4", [128, 512], f32, kind="ExternalInput").ap()
    md = nc.dram_tensor("mask", [2, 128, DIM], f32, kind="ExternalInput").ap()
    td = nc.dram_tensor("tempv", [2, 128, 1], f32, kind="ExternalInput").ap()
    od = nc.dram_tensor("out", [BL, DIM, M], f16, kind="ExternalOutput").ap()

    with TC(nc) as tc, ExitStack() as ctx:
        P = lambda **kw: ctx.enter_context(tc.tile_pool(**kw))
        consts = P(name="consts", bufs=1)
        p_qp = P(name="p_qp", bufs=2)
        p_x = P(name="p_x", bufs=2)
        p_qk = P(name="p_qk", bufs=4)
        p_sm = P(name="p_sm", bufs=2)
        p_tn = P(name="p_tn", bufs=4)
        p_fin = P(name="p_fin", bufs=6)
        # global PSUM pools: 4+1+1+2 = 8 banks exactly
        pp_ik = P(name="pp_ik", bufs=4, space="PSUM")
        pp_sc = P(name="pp_sc", bufs=1, space="PSUM")
        pp_g4 = P(name="pp_g4", bufs=1, space="PSUM")
        pp_pq = P(name="pp_pq", bufs=2, space="PSUM")

        # ---- constants; the conv weights are split across all three DMA
        # queues (behind each queue's first image chunk) so the first conv
        # tile can start ~2.5us in; softmax/out-chain consts are emitted
        # after batch 0's loads ----
        wdw8 = consts.tile([128, 2, 9 * DIM], f8, tag="wdw8", name="wdw8")
        wk8 = consts.tile([128, 2, DIM], f8, tag="wk8", name="wk8")

        def early_consts():
            for dy, eng in ((0, nc.scalar), (1, nc.sync), (2, nc.gpsimd)):
                c0, c1 = 3 * DIM * dy, 3 * DIM * (dy + 1)
                eng.dma_start(out=wdw8[:, :, c0:c1], in_=wdd[:, :, c0:c1])
            nc.scalar.dma_start(out=wk8[:], in_=wkd)
            # dummy activation primes the copy/exp/ln table while the image
            # DMAs land, so the first real ACT copy doesn't pay the ~1.3us
            # table load mid-stream
            nc.scalar.copy(scr[:], ones_row[:])
        wv2 = [consts.tile([128, DIM], f16, tag=f"wv2{k}", name=f"wv2{k}") for k in range(C128)]
        wp = [consts.tile([128, DIM], f32r, tag=f"wp{k}", name=f"wp{k}") for k in range(C128)]
        eye4 = consts.tile([128, 512], f32, tag="eye4", name="eye4")
        msk = [consts.tile([128, DIM], f32, tag=f"msk{k}", name=f"msk{k}") for k in range(2)]
        tmpv = [consts.tile([128, 1], f32, tag=f"tmpv{k}", name=f"tmpv{k}") for k in range(2)]

        def late_consts():
            for k in range(C128):
                sl = slice(128 * k, 128 * (k + 1))
                nc.sync.dma_start(out=wv2[k][:], in_=wvd[sl, :])
                nc.sync.dma_start(out=wp[k][:], in_=wpd[sl, :])
            nc.sync.dma_start(out=eye4[:], in_=eyd)
            for rr in range(2):
                nc.sync.dma_start(out=msk[rr][:], in_=md[rr])
                nc.sync.dma_start(out=tmpv[rr][:], in_=td[rr])
        ident = consts.tile([128, 128], f32, tag="ident", name="ident")
        make_identity(nc, ident[:])
        ones_row = consts.tile([1, 128], f32r, tag="ones", name="ones")
        onesf = consts.tile([1, 128], f32, tag="onesf", name="onesf")
        nc.vector.memset(onesf[:], 1.0)
        nc.vector.tensor_copy(ones_row[:], onesf[:])
        scr = consts.tile([1, 128], f32, tag="scr", name="scr")
        ones_c16 = consts.tile([128, 1], f16, tag="ones16", name="ones16")
        nc.vector.memset(ones_c16[:], 1.0)

        state = defaultdict(dict)

        def emit_load_q1(vk, b, after_c0=None):
            s = state[vk]
            s["b"] = b
            s["x8"] = p_x.tile([128, 2, M], f8, tag="x8", name="x8")
            # the q path input is 9 host-precomputed tap images, already
            # snake+interp-transformed to the m=3072 output positions; each
            # tap is two m-half tiles (whole-tile deps gate per tile).
            # batch 0 spreads firsts over all three queues for startup; later
            # batches avoid the ACT queue (it carries the PSUM copies)
            qa = [[p_qp.tile([128, 2, M // 2], f8, tag=f"qa{t}{h}",
                             name=f"qa{t}{h}") for h in range(2)]
                  for t in range(9)]
            s["qa"] = qa
            engs = (nc.sync, nc.scalar, nc.gpsimd)
            ne = len(engs)
            for h in range(2):
                for t in range(9):
                    engs[(h * 9 + t) % ne].dma_start(
                        out=qa[t][h][:],
                        in_=qad[b, t][:, :, 1536 * h:1536 * (h + 1)])
                if h == 0:
                    if after_c0 is not None:
                        after_c0()
                    # x8's first chunk before the second image halves: the
                    # first mtile's kT needs it right after the first images
                    nc.sync.dma_start(out=s["x8"][:, :, 0:1536],
                                      in_=x8d[b][:, :, 0:1536])
            nc.sync.dma_start(out=s["x8"][:, :, 1536:M],
                              in_=x8d[b][:, :, 1536:M])
            # e5m2 residual of x (for the fp8 error-feedback output matmul)
            s["x5"] = p_x.tile([128, 2, M], f8e5, tag="x5", name="x5")
            nc.gpsimd.dma_start(out=s["x5"][:], in_=x5d[b])

        def emit_stream(vk, hooks=()):
            s = state[vk]
            qa, x8t = s["qa"], s["x8"]
            ps_scc = pp_sc.tile([128, 512], f32, tag="pscc", name="pscc")
            ps_sc = [ps_scc[:, 0:DIM], ps_scc[:, DIM:512]]
            ps_g4 = pp_g4.tile([128, 512], f32, tag="g4", name="g4")
            s["sc"], s["scc"], s["g4"] = ps_sc, ps_scc, ps_g4
            qps = s["qps"] = []

            def emit_mtile(j):
                h, off = divmod(128 * j, M // 2)
                psik = pp_ik.tile([128, 512], f32, tag="pik", name="pik")
                # q = sum over 9 taps of (tap image @ folded weights), direct
                # at the 3072 interp positions; kT into [256:512] of the bank
                for t in range(9):
                    nc.tensor.matmul(
                        psik[:, 0:DIM], qa[t][h][:, :, off:off + 128],
                        wdw8[:, :, t * DIM:(t + 1) * DIM],
                        start=(t == 0), stop=False,
                        perf_mode=DR, skip_group_check=True)
                nc.tensor.matmul(
                    psik[:, DIM:512], x8t[:, :, 128 * j:128 * (j + 1)], wk8[:],
                    start=False, stop=True, perf_mode=DR, skip_group_check=True)
                if j % 2 == 0:
                    qps.append(p_qk.tile([128, 2, 512], f8, tag="qk8", name="qk8"))
                    nc.vector.tensor_copy(qps[-1][:, 0, :], psik[:])
                else:
                    nc.scalar.copy(qps[-1][:, 1, :], psik[:])

            def emit_scores(p):
                # lagged a few mtiles so the qk8 copies are long done
                qp = qps[p]
                for rr in range(2):
                    nc.tensor.matmul(
                        ps_sc[rr], qp[:, :, 128 * rr:128 * (rr + 1)],
                        qp[:, :, DIM:512],
                        start=(p == 0 and rr == 0),
                        stop=(p == NMT // 2 - 1 and rr == 1),
                        perf_mode=DR, skip_group_check=True)
                # gram blocks for l2 norms (diag extracted in softmax)
                for g in range(4):
                    nc.tensor.matmul(
                        ps_g4[:, 128 * g:128 * (g + 1)],
                        qp[:, :, 128 * g:128 * (g + 1)],
                        qp[:, :, 128 * g:128 * (g + 1)],
                        start=(p == 0), stop=(p == NMT // 2 - 1),
                        perf_mode=DR, skip_group_check=True)

            ndone = 0
            for j in range(NMT):
                emit_mtile(j)
                if j % 3 == 2 and j // 3 < len(hooks):
                    hooks[j // 3]()
                ready = max(0, (j - 3) // 2)
                while ndone < ready:
                    emit_scores(ndone)
                    ndone += 1

            def tail_scores(nd=ndone):
                for p in range(nd, NMT // 2):
                    emit_scores(p)

            # defer the last score pairs: their qk8 copies were just issued,
            # so the caller schedules them under later PE work
            s["tail_fn"] = tail_scores

        def softmax_parts(vk):
            """Softmax chain as three emission hooks, interleavable with the
            next batch's conv groups so the chain latency hides under them.
            Part 1 frees the pscc/g4 PSUM banks early (SBUF copies)."""
            s = state[vk]
            rqT, rZ = [], []

            def part1():
                s["tail_fn"]()  # this batch's deferred score pairs
                ps_g4 = s["g4"]
                # scores to SBUF, freeing the bank for the next stream
                scp = p_sm.tile([128, 512], f32, tag="scp", name="scp")
                s["scp"] = scp
                nc.scalar.copy(scp[:], s["scc"][:])
                # gram diagonals via eye mask
                geye = p_sm.tile([128, 512], f16, tag="geye", name="geye")
                nc.vector.tensor_mul(geye[:], ps_g4[:], eye4[:])
                # q norms straight to per-partition columns via DVE reduce
                # (skips the ones-matmul + PE transposes on the q side);
                # 1/sqrt as exp(-ln/2) keeps all ACT funcs in one table
                nq2 = p_tn.tile([128, 2], f32, tag="nq2", name="nq2")
                nc.vector.tensor_reduce(nq2[:, 0:1], geye[:, 0:128],
                                        axis=AxX, op=mybir.AluOpType.add)
                nc.vector.tensor_reduce(nq2[:, 1:2], geye[:, 128:256],
                                        axis=AxX, op=mybir.AluOpType.add)
                rq2 = p_tn.tile([128, 2], f32, tag="rq2", name="rq2")
                nc.scalar.activation(rq2[:], nq2[:], Ln)
                nc.scalar.activation(rq2[:], rq2[:], Exp, scale=-0.5)
                for rr in range(2):
                    rqt = p_tn.tile([128, 1], f32, tag="rqt", name="rqt")
                    nc.vector.tensor_mul(rqt[:], rq2[:, rr:rr + 1], tmpv[rr][:])
                    rqT.append(rqt)
                # k norms as a row (for the partition-broadcast matmul)
                ps_nqk = pp_pq.tile([1, DIM], f32, tag="pq", name="pq")
                nc.tensor.matmul(ps_nqk[:], ones_c16[:], geye[:, DIM:512],
                                 start=True, stop=True)
                rrow = p_sm.tile([1, DIM], f32, tag="rrow", name="rrow", bufs=2)
                nc.scalar.activation(rrow[:], ps_nqk[:], Ln)
                nc.scalar.activation(rrow[:], rrow[:], Exp, scale=-0.5)
                # DVE copy rounds to f32r so the broadcast matmul runs 4x
                # faster than plain f32
                rrow_r = p_sm.tile([1, DIM], f32r, tag="rrowr", name="rrowr",
                                   bufs=2)
                s["rrow"] = rrow_r
                nc.vector.tensor_copy(rrow_r[:], rrow[:])

            def part2():
                rrow = s["rrow"]
                # rnk broadcast down partitions via outer product; the sc
                # chain reads it straight from PSUM (no staging copy)
                psb = pp_pq.tile([128, DIM], f32, tag="pq", name="pq")
                s["psb"] = psb
                nc.tensor.matmul(psb[:], ones_row[:], rrow[:],
                                 start=True, stop=True)

            def part3():
                scp, rkb = s["scp"], s["psb"]
                E = [p_sm.tile([128, DIM], f32, tag="e", name="e") for _ in range(2)]
                # masked softmax, exp with fused row-sum; the all-SBUF chain
                # ops legally run on GPSIMD/Pool, freeing DVE for PSUM copies
                for rr in range(2):
                    sc = p_sm.tile([128, DIM], f32, tag="sc", name="sc")
                    # fused (scp * rq) * rkb in one DVE pass
                    nc.vector.scalar_tensor_tensor(
                        sc[:], scp[:, DIM * rr:DIM * (rr + 1)], rqT[rr][:],
                        rkb[:], op0=mybir.AluOpType.mult,
                        op1=mybir.AluOpType.mult)
                    nc.gpsimd.tensor_add(sc[:], sc[:], msk[rr][:])
                    z = p_tn.tile([128, 1], f32, tag="z", name="z")
                    nc.scalar.activation(E[rr][:], sc[:], Exp, accum_out=z[:])
                    rz = p_tn.tile([128, 1], f32, tag="rz", name="rz")
                    nc.vector.reciprocal(rz[:], z[:])
                    rZ.append(rz)
                # Ahat = E / Z (rows)
                Ahat = [p_sm.tile([128, DIM], f32r, tag="ah", name="ah", bufs=3)
                        for _ in range(2)]
                for rr in range(2):
                    nc.vector.tensor_scalar_mul(Ahat[rr][:], E[rr][:], rZ[rr][:])
                s["Ahat"] = Ahat

            return (part1, part2, part3)

        def emit_softmax(vk):
            for p in softmax_parts(vk):
                p()

        def out_head(vk):
            s = state[vk]
            Ahat = s["Ahat"]
            # m1t[d,o] = (W_po @ Ahat)^T ; wch[c,o] = (W_po @ Ahat @ W_v)^T
            m1t = [p_sm.tile([128, DIM], f16, tag="m1t", name="m1t") for _ in range(2)]
            for d in range(2):
                ps = pp_pq.tile([128, DIM], f32, tag="pq", name="pq")
                for k in range(C128):
                    nc.tensor.matmul(
                        ps[:], r(Ahat[k][:, 128 * d:128 * (d + 1)]), r(wp[k][:]),
                        start=(k == 0), stop=(k == C128 - 1))
                nc.scalar.copy(m1t[d][:], ps[:])
            # wch in e4m3 + e5m2 residual: the final matmul runs as fp8
            # DoubleRow with error feedback (wch8@x8 + wch8@dx + dwch@x8),
            # adding only ~0.2% output error but halving the PE cost
            wch8 = p_sm.tile([128, 2, DIM], f8, tag="wch8", name="wch8")
            dwch5 = p_sm.tile([128, 2, DIM], f8e5, tag="dwch5", name="dwch5")
            s["wch8"], s["dwch5"] = wch8, dwch5
            for cb in range(2):
                ps = pp_pq.tile([128, DIM], f32, tag="pq", name="pq")
                for d in range(2):
                    nc.tensor.matmul(
                        ps[:], wv2[d][:, 128 * cb:128 * (cb + 1)], m1t[d][:],
                        start=(d == 0), stop=(d == 1))
                nc.scalar.copy(wch8[:, cb, :], ps[:])
                nc.vector.tensor_sub(dwch5[:, cb, :], ps[:], wch8[:, cb, :])

        def fin_tile(vk, i, mixed=True):
            """One 1024-wide output tile of W_chain @ x (fp8 DoubleRow with
            error feedback). mixed=False keeps PSUM in the pq pool so tiles
            can interleave with a stream (whose psik owns the ik pool)."""
            s = state[vk]
            b = s["b"]
            x8t, x5t = s["x8"], s["x5"]
            wch8, dwch5 = s["wch8"], s["dwch5"]
            o, h = divmod(i, NM512 // 2)
            st = p_fin.tile([128, 1024], f16, tag="fin", name="fin")
            for half in range(2):
                n = 2 * h + half
                pool = pp_pq if (not mixed or (2 * i + half) % 2 == 0) else pp_ik
                tg = "pq" if pool is pp_pq else "pik"
                ps = pool.tile([128, 512], f32, tag=tg, name=tg)
                osl = slice(128 * o, 128 * (o + 1))
                nsl = slice(512 * n, 512 * (n + 1))
                nc.tensor.matmul(ps[:], wch8[:, :, osl], x8t[:, :, nsl],
                                 start=True, stop=False, perf_mode=DR)
                nc.tensor.matmul(ps[:], wch8[:, :, osl], x5t[:, :, nsl],
                                 start=False, stop=False, perf_mode=DR)
                nc.tensor.matmul(ps[:], dwch5[:, :, osl], x8t[:, :, nsl],
                                 start=False, stop=True, perf_mode=DR)
                if half == 0:
                    nc.scalar.copy(st[:, 0:512], ps[:])
                else:
                    nc.vector.tensor_copy(st[:, 512:1024], ps[:])
            deng = nc.sync if h % 2 == 0 else nc.gpsimd
            deng.dma_start(
                out=od[b, 128 * o:128 * (o + 1), 1024 * h:1024 * (h + 1)],
                in_=st[:])

        def fin_tile_split(vk, i):
            # last tile of the kernel: two half staging tiles with parallel
            # DMAs on separate queues shortens the post-matmul drain chain
            s = state[vk]
            b = s["b"]
            x8t, x5t = s["x8"], s["x5"]
            wch8, dwch5 = s["wch8"], s["dwch5"]
            o, h = divmod(i, NM512 // 2)
            for half in range(2):
                n = 2 * h + half
                pool = pp_pq if half == 0 else pp_ik
                tg = "pq" if pool is pp_pq else "pik"
                ps = pool.tile([128, 512], f32, tag=tg, name=tg)
                osl = slice(128 * o, 128 * (o + 1))
                nsl = slice(512 * n, 512 * (n + 1))
                nc.tensor.matmul(ps[:], wch8[:, :, osl], x8t[:, :, nsl],
                                 start=True, stop=False, perf_mode=DR)
                nc.tensor.matmul(ps[:], wch8[:, :, osl], x5t[:, :, nsl],
                                 start=False, stop=False, perf_mode=DR)
                nc.tensor.matmul(ps[:], dwch5[:, :, osl], x8t[:, :, nsl],
                                 start=False, stop=True, perf_mode=DR)
                st = p_fin.tile([128, 512], f16, tag="fin2", name="fin2")
                if half == 0:
                    nc.scalar.copy(st[:], ps[:])
                else:
                    nc.vector.tensor_copy(st[:], ps[:])
                deng = nc.sync if half == 0 else nc.gpsimd
                deng.dma_start(
                    out=od[b, 128 * o:128 * (o + 1), 512 * n:512 * (n + 1)],
                    in_=st[:])

        def emit_out(vk, hooks=(), tiles=None, split_last=False):
            s = state[vk]
            if "wch8" not in s:
                out_head(vk)
            if len(hooks) > 0:
                hooks[0]()
            idxs = list(range(2 * (NM512 // 2)) if tiles is None else tiles)
            for i in idxs:
                if split_last and i == idxs[-1]:
                    fin_tile_split(vk, i)
                else:
                    fin_tile(vk, i)
                if i + 1 < len(hooks):
                    hooks[i + 1]()

        # software pipeline: q1(b+1) fills the PE while batch b's softmax
        # chain runs *inside* stream(b+1) via hooks (its tiny PE ops slot
        # between conv groups; part 1 frees the score/gram PSUM banks before
        # stream(b+1)'s first scores matmul needs them)
        sq_ = list(range(n_batches)) if seq is None else list(seq)
        vis = [(i, b) for i, b in enumerate(sq_)]
        n = len(vis)
        emit_load_q1(0, vis[0][1], after_c0=early_consts)
        emit_stream(0)
        late_consts()
        for i in range(1, n):
            emit_load_q1(i, vis[i][1])
            # softmax(i-1) and batch i-1's out-head hide inside stream(i)'s
            # conv groups
            sm = softmax_parts(i - 1)
            emit_stream(i, hooks=(*sm, lambda vv=i - 1: out_head(vv)))
            if i >= 2:
                emit_out(i - 2)
        if n > 1:
            # tail: last softmax chain hides under out(n-2)'s fin stream
            emit_out(n - 2, hooks=softmax_parts(n - 1))
            out_head(n - 1)
            emit_out(n - 1, split_last=True)
        else:
            emit_softmax(0)
            emit_out(0)

    if split_waits:
        _split_waits(nc)
    return nc


def _get_nc():
    if "nc" not in _CACHE:
        _CACHE["nc"] = build_nc()
    return _CACHE["nc"]


def make_inputs(inputs):
    """Host-side prep: consts + per-core sharded activations."""
    consts = _host_consts(inputs["W_kv"], inputs["W_q"], inputs["W_dw"],
                          inputs["W_po"], inputs["temperature"])
    xf = np.asarray(inputs["x"], np.float32)
    x8f = _f8(xf)
    x5f = _f8e5(xf - np.float32(x8f))
    x8 = x8f.reshape(B, 2, 128, M).transpose(0, 2, 1, 3)
    x5 = x5f.reshape(B, 2, 128, M).transpose(0, 2, 1, 3)
    qa8 = _f8(_tap_images(inputs["y"])).reshape(B, 9, 2, 128, M)
    qa8 = qa8.transpose(0, 1, 3, 2, 4)
    in_maps = []
    for i in range(NCORES):
        m = dict(consts)
        m["x8_sh"] = np.ascontiguousarray(x8[BL * i:BL * (i + 1)])
        m["x5_sh"] = np.ascontiguousarray(x5[BL * i:BL * (i + 1)])
        m["qa8_sh"] = np.ascontiguousarray(qa8[BL * i:BL * (i + 1)])
        in_maps.append(m)
    return in_maps


def run(inputs, trace=False, trace_kwargs=None):
    from concourse.bass_utils import run_bass_kernel_spmd

    nc = _get_nc()
    in_maps = make_inputs(inputs)
    res = run_bass_kernel_spmd(
        nc, in_maps, core_ids=list(range(NCORES)), trace=trace,
        trace_kwargs=trace_kwargs or {})
    out = np.concatenate(
        [np.asarray(res.results[i]["out"], np.float32) for i in range(NCORES)],
        axis=0)
    return out, res


def kernel(**inputs) -> np.ndarray:
    out, _ = run(inputs, trace=False)
    return out


# revision 122
# speedup vs baseline: 1.0334x; 1.0045x over previous
"""Trainium2 Bass kernel for nn_Attention1 (channel attention transformer block).

Reference computation (per batch):
  kv = W_kv @ x ; k, v = split(kv)                    # pointwise conv over m=3072
  q  = conv3x3(W_q @ y, W_dw)                         # 1x1 then full 3x3, 64x64 image
  q  = linear_interp(snake(q.flatten(HW)), 4096->3072)
  q, k = l2norm over m ; attn = softmax(q @ k^T * temp) per 32-channel head
  out = W_po @ (attn @ v)

Sharding: data-parallel over batch, 16 batches / 8 cores = 2 per core. SPMD,
no collectives; per-core outputs are concatenated on host.

Per-core kernel layout strategy. All heavy matmuls run in fp8(e4m3) with
perf_mode=DoubleRow: operands are laid out [128 partitions, 2, cols] so one
matmul contracts 256 channels (the pair dim sums in the PE), doubling PE
throughput vs fp16. fp8 is safe for everything that feeds the l2-normalized
q/k (scale and elementwise quantization wash out to ~0.1% at score level);
the final W_chain @ x matmul stays fp16 since it writes the output directly.
  - q path   : y kept as fp8 chan-pair images, zero-padded 66x64, three
               horizontally shifted copies; 3x3 conv as 9 DoubleRow matmuls
               (image stationary), both 128-chan blocks contracted at once.
               conv outputs for two row-tiles share one PSUM bank, one copy
               into a contiguous fp8 ct arena (32 x [128,256] slots)
  - snake+interp : fused sparse matrix S applied via one DoubleRow matmul
               per m-tile (S blocks repeat with period 3; the two
               contributing 128x128 blocks ride the pair dim)
  - kT       : x chan-pair fp8 stationary, W_k^T pair fp8 moving -> kT
               (m on partitions), same PSUM bank as interp (one copy)
  - scores   : qk tiles copied to fp8 pair arenas (two m-tiles per pair);
               q@kT via DoubleRow contracting 256 m-rows per matmul
  - norms    : gram blocks qk8[s]^T qk8[s] accumulate in PSUM; diagonal
               extracted with an eye mask (DVE) + ones-matmul -> packed
               [1,512] row of |q|^2,|k|^2 (replaces explicit squares)
  - softmax  : per-head masking via additive -30 mask over the full 256-wide
               score rows; exp on ScalarE with fused row-sum (accum_out);
               1/Z folded into Ahat as a per-row scale
  - out      : W_po @ A @ W_v folded into a 256x256 chain (f32r/f16 small
               matmuls), then W_chain @ x streamed in fp16; output DMA'd
               as f16 and upcast on host
"""
import numpy as np

HEADS = 8
B, DIM, M = 16, 256, 3072
HW = 64
L = HW * HW          # 4096 flattened conv spatial size
NCORES = 8
BL = B // NCORES     # batches per core
C128 = DIM // 128    # channel 128-tiles (2)
NM512 = M // 512     # m-dim 512-tiles (6)
NMT = M // 128       # m-dim 128-tiles (24)
NST = L // 128       # conv-spatial 128-tiles (32)

_CACHE = {}


def _f8(a):
    import ml_dtypes
    return np.asarray(a, np.float32).astype(ml_dtypes.float8_e4m3)


def _f8e5(a):
    import ml_dtypes
    return np.asarray(a, np.float32).astype(ml_dtypes.float8_e5m2)


def _tap_images(y):
    """Fuse conv-shift + snake + linear-interp on the host: for each of the
    9 conv taps, gather the two interp source pixels of each of the m=3072
    output positions from the zero-padded shifted image. The on-chip q path
    is then just 9 DoubleRow matmuls per m-tile against the folded weights
    (computing q directly at 3072 positions instead of 4096 + interp)."""
    mask = np.arange(L).reshape(HW, HW)
    mask[1::2] = mask[1::2][:, ::-1]
    mask = mask.reshape(-1)
    srcp = (np.arange(M) + 0.5) * (L / M) - 0.5
    srcp = np.maximum(srcp, 0.0)
    i0 = np.minimum(np.floor(srcp).astype(np.int64), L - 1)
    i1 = np.minimum(i0 + 1, L - 1)
    lam = (srcp - i0).astype(np.float32)
    g0, g1 = mask[i0], mask[i1]
    y = np.asarray(y, np.float32)
    ypad = np.zeros((B, DIM, HW + 2, HW + 2), np.float32)
    ypad[:, :, 1:HW + 1, 1:HW + 1] = y
    out = np.empty((B, 9, DIM, M), np.float32)
    for dy in range(3):
        for dx in range(3):
            sh = ypad[:, :, dy:dy + HW, dx:dx + HW].reshape(B, DIM, L)
            out[:, dy * 3 + dx] = sh[:, :, g0] * (1 - lam) + sh[:, :, g1] * lam
    return out


def _host_consts(W_kv, W_q, W_dw, W_po, temperature):
    c = np.arange(DIM)
    mask = np.where((c[:, None] // 32) == (c[None, :] // 32), 0.0, -30.0).astype(np.float32)
    tv = np.repeat(np.asarray(temperature, np.float32).reshape(HEADS), DIM // HEADS)
    # folded conv weights: (W_dw . W_q) -> [in-chan b, dy, dx, out o], then
    # chan-pair layout [128 p, 2 t, 9*256] with in-chan = p + 128 t
    fold = np.einsum("oayx,ab->byxo", np.asarray(W_dw, np.float32),
                     np.asarray(W_q, np.float32))
    wdw8 = fold.reshape(2, 128, 9 * DIM).transpose(1, 0, 2)
    # W_k^T chan-pair: [128 p, 2 t, 256 o] with in-chan = p + 128 t
    wk8 = W_kv[:DIM].T.reshape(2, 128, DIM).transpose(1, 0, 2)
    eye4 = np.concatenate([np.eye(128, dtype=np.float32)] * 4, axis=1)
    return {
        "w_k8": _f8(wk8),
        "w_v2": np.ascontiguousarray(W_kv[DIM:], np.float16),
        "w_poT": np.ascontiguousarray(W_po.T, np.float32),
        "w_dw8": _f8(wdw8),
        "eye4": np.ascontiguousarray(eye4),
        "mask": np.ascontiguousarray(mask.reshape(2, 128, DIM)),
        "tempv": np.ascontiguousarray(tv.reshape(2, 128, 1)),
    }


def _make_tc_class():
    """TileContext subclass splitting the end-of-kernel drain waits.

    This container's walrus rejects >1 sem wait on CTRL-encoded instructions
    (Drain/NoOp). The stock Tile epilogue hangs every semaphore's final value
    on one Drain. Emit a chain of SP NoOps with one wait each instead, then a
    waitless drain: SP reaches it only after all sems hit their final values.
    """
    import bass_rust
    import concourse.mybir as mybir
    import concourse.tile as tile

    class SplitDrainTileContext(tile.TileContext):
        def _drain_and_barrier(self, tick_clock, wait_clock):
            probe = self.nc.sync.nop()
            wait_clock.add_sem_waits(
                probe.ins, bass_rust.ScopedClock({None: tick_clock.global_clock})
            )
            waits = list(probe.ins.sync_info.on_wait or [])
            probe.ins.sync_info.on_wait = waits[:1]
            for w in waits[1:]:
                n2 = self.nc.sync.nop()
                n2.ins.sync_info = mybir.SyncInfo(on_wait=[w], on_update=[])
            self.nc.sync.drain()
            self.nc.all_engine_barrier()
            assert self.sems is not None
            popped = self.nc._tile_sem_poison_stack.pop()
            assert popped is self._sem_poison
            self.nc.clear_and_free_semaphores(list(self.sems.allocated().values()))
            self.nc.all_engine_barrier()

    return SplitDrainTileContext


def _split_waits(nc):
    """Walrus in this container allows only one sem wait per instruction.
    Move extra waits onto same-engine NoOps inserted just before."""
    import concourse.mybir as mybir
    n = 0
    for f in nc.m.functions:
        for bb in f.blocks:
            out = []
            changed = False
            for inst in bb.instructions:
                si = inst.sync_info
                waits = list(si.on_wait) if si and si.on_wait else []
                if len(waits) > 1:
                    for w in waits[:-1]:
                        n += 1
                        nop = mybir.InstNoOp(name=f"I-sw{n}-{inst.name}", ins=[], outs=[])
                        nop.engine = inst.engine
                        nop.sync_info = mybir.SyncInfo(on_wait=[w], on_update=[])
                        out.append(nop)
                    si.on_wait = [waits[-1]]
                    changed = True
                out.append(inst)
            if changed:
                bb.instructions = out
    return n


def build_nc(split_waits=True, n_batches=BL, seq=None):
    from contextlib import ExitStack
    from collections import defaultdict
    import concourse.bass as bass
    import concourse.mybir as mybir
    from concourse.masks import make_identity

    f32 = mybir.dt.float32
    f32r = mybir.dt.float32r
    u32 = mybir.dt.uint32
    u8 = mybir.dt.uint8
    f16 = mybir.dt.float16
    f8 = mybir.dt.float8e4
    DR = mybir.MatmulPerfMode.DoubleRow
    Exp = mybir.ActivationFunctionType.Exp
    Ln = mybir.ActivationFunctionType.Ln
    AxX = mybir.AxisListType.X

    def r(ap):
        return ap.bitcast(f32r)

    TC = _make_tc_class()
    nc = bass.Bass("TRN2", target_bir_lowering=False, debug=False)

    f8e5 = mybir.dt.float8e5
    x8d = nc.dram_tensor("x8_sh", [BL, 128, 2, M], f8, kind="ExternalInput").ap()
    x5d = nc.dram_tensor("x5_sh", [BL, 128, 2, M], f8e5, kind="ExternalInput").ap()
    qad = nc.dram_tensor("qa8_sh", [BL, 9, 128, 2, M], f8, kind="ExternalInput").ap()
    wkd = nc.dram_tensor("w_k8", [128, 2, DIM], f8, kind="ExternalInput").ap()
    wvd = nc.dram_tensor("w_v2", [DIM, DIM], f16, kind="ExternalInput").ap()
    wpd = nc.dram_tensor("w_poT", [DIM, DIM], f32r, kind="ExternalInput").ap()
    wdd = nc.dram_tensor("w_dw8", [128, 2, 9 * DIM], f8, kind="ExternalInput").ap()
    eyd = nc.dram_tensor("eye4", [128, 512], f32, kind="ExternalInput").ap()
    md = nc.dram_tensor("mask", [2, 128, DIM], f32, kind="ExternalInput").ap()
    td = nc.dram_tensor("tempv", [2, 128, 1], f32, kind="ExternalInput").ap()
    od = nc.dram_tensor("out", [BL, DIM, M], f16, kind="ExternalOutput").ap()

    with TC(nc) as tc, ExitStack() as ctx:
        P = lambda **kw: ctx.enter_context(tc.tile_pool(**kw))
        consts = P(name="consts", bufs=1)
        p_qp = P(name="p_qp", bufs=2)
        p_x = P(name="p_x", bufs=2)
        p_qk = P(name="p_qk", bufs=4)
        p_sm = P(name="p_sm", bufs=2)
        p_tn = P(name="p_tn", bufs=4)
        p_fin = P(name="p_fin", bufs=6)
        # global PSUM pools: 4+1+1+2 = 8 banks exactly
        pp_ik = P(name="pp_ik", bufs=4, space="PSUM")
        pp_sc = P(name="pp_sc", bufs=1, space="PSUM")
        pp_g4 = P(name="pp_g4", bufs=1, space="PSUM")
        pp_pq = P(name="pp_pq", bufs=2, space="PSUM")

        # ---- constants; the conv weights are split across all three DMA
        # queues (behind each queue's first image chunk) so the first conv
        # tile can start ~2.5us in; softmax/out-chain consts are emitted
        # after batch 0's loads ----
        wdw8 = consts.tile([128, 2, 9 * DIM], f8, tag="wdw8", name="wdw8")
        wk8 = consts.tile([128, 2, DIM], f8, tag="wk8", name="wk8")

        def early_consts():
            for dy, eng in ((0, nc.scalar), (1, nc.sync), (2, nc.gpsimd)):
                c0, c1 = 3 * DIM * dy, 3 * DIM * (dy + 1)
                eng.dma_start(out=wdw8[:, :, c0:c1], in_=wdd[:, :, c0:c1])
            nc.scalar.dma_start(out=wk8[:], in_=wkd)
            # dummy activation primes the copy/exp/ln table while the image
            # DMAs land, so the first real ACT copy doesn't pay the ~1.3us
            # table load mid-stream
            nc.scalar.copy(scr[:], ones_row[:])
        wv2 = [consts.tile([128, DIM], f16, tag=f"wv2{k}", name=f"wv2{k}") for k in range(C128)]
        wp = [consts.tile([128, DIM], f32r, tag=f"wp{k}", name=f"wp{k}") for k in range(C128)]
        eye4 = consts.tile([128, 512], f32, tag="eye4", name="eye4")
        msk = [consts.tile([128, DIM], f32, tag=f"msk{k}", name=f"msk{k}") for k in range(2)]
        tmpv = [consts.tile([128, 1], f32, tag=f"tmpv{k}", name=f"tmpv{k}") for k in range(2)]

        def late_consts():
            for k in range(C128):
                sl = slice(128 * k, 128 * (k + 1))
                nc.sync.dma_start(out=wv2[k][:], in_=wvd[sl, :])
                nc.sync.dma_start(out=wp[k][:], in_=wpd[sl, :])
            nc.sync.dma_start(out=eye4[:], in_=eyd)
            for rr in range(2):
                nc.sync.dma_start(out=msk[rr][:], in_=md[rr])
                nc.sync.dma_start(out=tmpv[rr][:], in_=td[rr])
        ident = consts.tile([128, 128], f32, tag="ident", name="ident")
        make_identity(nc, ident[:])
        ones_row = consts.tile([1, 128], f32r, tag="ones", name="ones")
        onesf = consts.tile([1, 128], f32, tag="onesf", name="onesf")
        nc.vector.memset(onesf[:], 1.0)
        nc.vector.tensor_copy(ones_row[:], onesf[:])
        scr = consts.tile([1, 128], f32, tag="scr", name="scr")
        ones_c16 = consts.tile([128, 1], f16, tag="ones16", name="ones16")
        nc.vector.memset(ones_c16[:], 1.0)

        state = defaultdict(dict)

        def emit_load_q1(vk, b, after_c0=None):
            s = state[vk]
            s["b"] = b
            s["x8"] = p_x.tile([128, 2, M], f8, tag="x8", name="x8")
            # the q path input is 9 host-precomputed tap images, already
            # snake+interp-transformed to the m=3072 output positions; each
            # tap is two m-half tiles (whole-tile deps gate per tile).
            # batch 0 spreads firsts over all three queues for startup; later
            # batches avoid the ACT queue (it carries the PSUM copies)
            qa = [[p_qp.tile([128, 2, M // 2], f8, tag=f"qa{t}{h}",
                             name=f"qa{t}{h}") for h in range(2)]
                  for t in range(9)]
            s["qa"] = qa
            engs = (nc.sync, nc.scalar, nc.gpsimd)
            ne = len(engs)
            for h in range(2):
                for t in range(9):
                    engs[(h * 9 + t) % ne].dma_start(
                        out=qa[t][h][:],
                        in_=qad[b, t][:, :, 1536 * h:1536 * (h + 1)])
                if h == 0:
                    if after_c0 is not None:
                        after_c0()
                    # x8's first chunk before the second image halves: the
                    # first mtile's kT needs it right after the first images
                    nc.sync.dma_start(out=s["x8"][:, :, 0:1536],
                                      in_=x8d[b][:, :, 0:1536])
            nc.sync.dma_start(out=s["x8"][:, :, 1536:M],
                              in_=x8d[b][:, :, 1536:M])
            # e5m2 residual of x (for the fp8 error-feedback output matmul)
            s["x5"] = p_x.tile([128, 2, M], f8e5, tag="x5", name="x5")
            nc.gpsimd.dma_start(out=s["x5"][:], in_=x5d[b])

        def emit_stream(vk, hooks=()):
            s = state[vk]
            qa, x8t = s["qa"], s["x8"]
            ps_scc = pp_sc.tile([128, 512], f32, tag="pscc", name="pscc")
            ps_sc = [ps_scc[:, 0:DIM], ps_scc[:, DIM:512]]
            ps_g4 = pp_g4.tile([128, 512], f32, tag="g4", name="g4")
            s["sc"], s["scc"], s["g4"] = ps_sc, ps_scc, ps_g4
            qps = s["qps"] = []

            def emit_mtile(j):
                h, off = divmod(128 * j, M // 2)
                psik = pp_ik.tile([128, 512], f32, tag="pik", name="pik")
                # q = sum over 9 taps of (tap image @ folded weights), direct
                # at the 3072 interp positions; kT into [256:512] of the bank
                for t in range(9):
                    nc.tensor.matmul(
                        psik[:, 0:DIM], qa[t][h][:, :, off:off + 128],
                        wdw8[:, :, t * DIM:(t + 1) * DIM],
                        start=(t == 0), stop=False,
                        perf_mode=DR, skip_group_check=True)
                nc.tensor.matmul(
                    psik[:, DIM:512], x8t[:, :, 128 * j:128 * (j + 1)], wk8[:],
                    start=False, stop=True, perf_mode=DR, skip_group_check=True)
                if j % 2 == 0:
                    qps.append(p_qk.tile([128, 2, 512], f8, tag="qk8", name="qk8"))
                    nc.vector.tensor_copy(qps[-1][:, 0, :], psik[:])
                else:
                    nc.scalar.copy(qps[-1][:, 1, :], psik[:])

            def emit_scores(p):
                # lagged a few mtiles so the qk8 copies are long done
                qp = qps[p]
                for rr in range(2):
                    nc.tensor.matmul(
                        ps_sc[rr], qp[:, :, 128 * rr:128 * (rr + 1)],
                        qp[:, :, DIM:512],
                        start=(p == 0 and rr == 0),
                        stop=(p == NMT // 2 - 1 and rr == 1),
                        perf_mode=DR, skip_group_check=True)
                # gram blocks for l2 norms (diag extracted in softmax)
                for g in range(4):
                    nc.tensor.matmul(
                        ps_g4[:, 128 * g:128 * (g + 1)],
                        qp[:, :, 128 * g:128 * (g + 1)],
                        qp[:, :, 128 * g:128 * (g + 1)],
                        start=(p == 0), stop=(p == NMT // 2 - 1),
                        perf_mode=DR, skip_group_check=True)

            ndone = 0
            for j in range(NMT):
                emit_mtile(j)
                if j % 3 == 2 and j // 3 < len(hooks):
                    hooks[j // 3]()
                ready = max(0, (j - 3) // 2)
                while ndone < ready:
                    emit_scores(ndone)
                    ndone += 1

            def tail_scores(nd=ndone):
                for p in range(nd, NMT // 2):
                    emit_scores(p)

            # defer the last score pairs: their qk8 copies were just issued,
            # so the caller schedules them under later PE work
            s["tail_fn"] = tail_scores

        def softmax_parts(vk):
            """Softmax chain as three emission hooks, interleavable with the
            next batch's conv groups so the chain latency hides under them.
            Part 1 frees the pscc/g4 PSUM banks early (SBUF copies)."""
            s = state[vk]
            rqT, rZ = [], []

            def part1():
                s["tail_fn"]()  # this batch's deferred score pairs
                ps_g4 = s["g4"]
                # scores to SBUF, freeing the bank for the next stream
                scp = p_sm.tile([128, 512], f32, tag="scp", name="scp")
                s["scp"] = scp
                nc.scalar.copy(scp[:], s["scc"][:])
                # gram diagonals via eye mask
                geye = p_sm.tile([128, 512], f16, tag="geye", name="geye")
                nc.vector.tensor_mul(geye[:], ps_g4[:], eye4[:])
                # q norms straight to per-partition columns via DVE reduce
                # (skips the ones-matmul + PE transposes on the q side);
                # 1/sqrt as exp(-ln/2) keeps all ACT funcs in one table
                nq2 = p_tn.tile([128, 2], f32, tag="nq2", name="nq2")
                nc.vector.tensor_reduce(nq2[:, 0:1], geye[:, 0:128],
                                        axis=AxX, op=mybir.AluOpType.add)
                nc.vector.tensor_reduce(nq2[:, 1:2], geye[:, 128:256],
                                        axis=AxX, op=mybir.AluOpType.add)
                rq2 = p_tn.tile([128, 2], f32, tag="rq2", name="rq2")
                nc.scalar.activation(rq2[:], nq2[:], Ln)
                nc.scalar.activation(rq2[:], rq2[:], Exp, scale=-0.5)
                for rr in range(2):
                    rqt = p_tn.tile([128, 1], f32, tag="rqt", name="rqt")
                    nc.vector.tensor_mul(rqt[:], rq2[:, rr:rr + 1], tmpv[rr][:])
                    rqT.append(rqt)
                # k norms as a row (for the partition-broadcast matmul)
                ps_nqk = pp_pq.tile([1, DIM], f32, tag="pq", name="pq")
                nc.tensor.matmul(ps_nqk[:], ones_c16[:], geye[:, DIM:512],
                                 start=True, stop=True)
                rrow = p_sm.tile([1, DIM], f32, tag="rrow", name="rrow", bufs=2)
                nc.scalar.activation(rrow[:], ps_nqk[:], Ln)
                nc.scalar.activation(rrow[:], rrow[:], Exp, scale=-0.5)
                # DVE copy rounds to f32r so the broadcast matmul runs 4x
                # faster than plain f32
                rrow_r = p_sm.tile([1, DIM], f32r, tag="rrowr", name="rrowr",
                                   bufs=2)
                s["rrow"] = rrow_r
                nc.vector.tensor_copy(rrow_r[:], rrow[:])

            def part2():
                rrow = s["rrow"]
                # rnk broadcast down partitions via outer product; the sc
                # chain reads it straight from PSUM (no staging copy)
                psb = pp_pq.tile([128, DIM], f32, tag="pq", name="pq")
                s["psb"] = psb
                nc.tensor.matmul(psb[:], ones_row[:], rrow[:],
                                 start=True, stop=True)

            def part3():
                scp, rkb = s["scp"], s["psb"]
                E = [p_sm.tile([128, DIM], f32, tag="e", name="e") for _ in range(2)]
                # masked softmax, exp with fused row-sum; the all-SBUF chain
                # ops legally run on GPSIMD/Pool, freeing DVE for PSUM copies
                for rr in range(2):
                    sc = p_sm.tile([128, DIM], f32, tag="sc", name="sc")
                    # fused (scp * rq) * rkb in one DVE pass
                    nc.vector.scalar_tensor_tensor(
                        sc[:], scp[:, DIM * rr:DIM * (rr + 1)], rqT[rr][:],
                        rkb[:], op0=mybir.AluOpType.mult,
                        op1=mybir.AluOpType.mult)
                    nc.gpsimd.tensor_add(sc[:], sc[:], msk[rr][:])
                    z = p_tn.tile([128, 1], f32, tag="z", name="z")
                    nc.scalar.activation(E[rr][:], sc[:], Exp, accum_out=z[:])
                    rz = p_tn.tile([128, 1], f32, tag="rz", name="rz")
                    nc.vector.reciprocal(rz[:], z[:])
                    rZ.append(rz)
                # Ahat = E / Z (rows)
                Ahat = [p_sm.tile([128, DIM], f32r, tag="ah", name="ah", bufs=3)
                        for _ in range(2)]
                for rr in range(2):
                    nc.vector.tensor_scalar_mul(Ahat[rr][:], E[rr][:], rZ[rr][:])
                s["Ahat"] = Ahat

            return (part1, part2, part3)

        def emit_softmax(vk):
            for p in softmax_parts(vk):
                p()

        def out_head(vk):
            s = state[vk]
            Ahat = s["Ahat"]
            # m1t[d,o] = (W_po @ Ahat)^T ; wch[c,o] = (W_po @ Ahat @ W_v)^T
            m1t = [p_sm.tile([128, DIM], f16, tag="m1t", name="m1t") for _ in range(2)]
            for d in range(2):
                ps = pp_pq.tile([128, DIM], f32, tag="pq", name="pq")
                for k in range(C128):
                    nc.tensor.matmul(
                        ps[:], r(Ahat[k][:, 128 * d:128 * (d + 1)]), r(wp[k][:]),
                        start=(k == 0), stop=(k == C128 - 1))
                nc.scalar.copy(m1t[d][:], ps[:])
            # wch in e4m3 + e5m2 residual: the final matmul runs as fp8
            # DoubleRow with error feedback (wch8@x8 + wch8@dx + dwch@x8),
            # adding only ~0.2% output error but halving the PE cost
            wch8 = p_sm.tile([128, 2, DIM], f8, tag="wch8", name="wch8")
            dwch5 = p_sm.tile([128, 2, DIM], f8e5, tag="dwch5", name="dwch5")
            s["wch8"], s["dwch5"] = wch8, dwch5
            for cb in range(2):
                ps = pp_pq.tile([128, DIM], f32, tag="pq", name="pq")
                for d in range(2):
                    nc.tensor.matmul(
                        ps[:], wv2[d][:, 128 * cb:128 * (cb + 1)], m1t[d][:],
                        start=(d == 0), stop=(d == 1))
                nc.scalar.copy(wch8[:, cb, :], ps[:])
                nc.vector.tensor_sub(dwch5[:, cb, :], ps[:], wch8[:, cb, :])

        def fin_tile(vk, i, mixed=True):
            """One 1024-wide output tile of W_chain @ x (fp8 DoubleRow with
            error feedback). mixed=False keeps PSUM in the pq pool so tiles
            can interleave with a stream (whose psik owns the ik pool)."""
            s = state[vk]
            b = s["b"]
            x8t, x5t = s["x8"], s["x5"]
            wch8, dwch5 = s["wch8"], s["dwch5"]
            o, h = divmod(i, NM512 // 2)
            st = p_fin.tile([128, 1024], f16, tag="fin", name="fin")
            for half in range(2):
                n = 2 * h + half
                pool = pp_pq if (not mixed or (2 * i + half) % 2 == 0) else pp_ik
                tg = "pq" if pool is pp_pq else "pik"
                ps = pool.tile([128, 512], f32, tag=tg, name=tg)
                osl = slice(128 * o, 128 * (o + 1))
                nsl = slice(512 * n, 512 * (n + 1))
                nc.tensor.matmul(ps[:], wch8[:, :, osl], x8t[:, :, nsl],
                                 start=True, stop=False, perf_mode=DR)
                nc.tensor.matmul(ps[:], wch8[:, :, osl], x5t[:, :, nsl],
                                 start=False, stop=False, perf_mode=DR)
                nc.tensor.matmul(ps[:], dwch5[:, :, osl], x8t[:, :, nsl],
                                 start=False, stop=True, perf_mode=DR)
                if half == 0:
                    nc.vector.tensor_copy(st[:, 0:512], ps[:])
                else:
                    nc.scalar.copy(st[:, 512:1024], ps[:])
            deng = nc.sync if h % 2 == 0 else nc.gpsimd
            deng.dma_start(
                out=od[b, 128 * o:128 * (o + 1), 1024 * h:1024 * (h + 1)],
                in_=st[:])

        def fin_tile_split(vk, i):
            # last tile of the kernel: two half staging tiles with parallel
            # DMAs on separate queues shortens the post-matmul drain chain
            s = state[vk]
            b = s["b"]
            x8t, x5t = s["x8"], s["x5"]
            wch8, dwch5 = s["wch8"], s["dwch5"]
            o, h = divmod(i, NM512 // 2)
            for half in range(2):
                n = 2 * h + half
                pool = pp_pq if half == 0 else pp_ik
                tg = "pq" if pool is pp_pq else "pik"
                ps = pool.tile([128, 512], f32, tag=tg, name=tg)
                osl = slice(128 * o, 128 * (o + 1))
                nsl = slice(512 * n, 512 * (n + 1))
                nc.tensor.matmul(ps[:], wch8[:, :, osl], x8t[:, :, nsl],
                                 start=True, stop=False, perf_mode=DR)
                nc.tensor.matmul(ps[:], wch8[:, :, osl], x5t[:, :, nsl],
                                 start=False, stop=False, perf_mode=DR)
                nc.tensor.matmul(ps[:], dwch5[:, :, osl], x8t[:, :, nsl],
                                 start=False, stop=True, perf_mode=DR)
                st = p_fin.tile([128, 512], f16, tag="fin2", name="fin2")
                if half == 0:
                    nc.scalar.copy(st[:], ps[:])
                else:
                    nc.vector.tensor_copy(st[:], ps[:])
                deng = nc.sync if half == 0 else nc.gpsimd
                deng.dma_start(
                    out=od[b, 128 * o:128 * (o + 1), 512 * n:512 * (n + 1)],
                    in_=st[:])

        def emit_out(vk, hooks=(), tiles=None, split_last=False):
            s = state[vk]
            if "wch8" not in s:
                out_head(vk)
            if len(hooks) > 0:
                hooks[0]()
            idxs = list(range(2 * (NM512 // 2)) if tiles is None else tiles)
            for i in idxs:
                if split_last and i == idxs[-1]:
                    fin_tile_split(vk, i)
                else:
                    fin_tile(vk, i)
                if i + 1 < len(hooks):
                    hooks[i + 1]()

        # software pipeline: q1(b+1) fills the PE while batch b's softmax
        # chain runs *inside* stream(b+1) via hooks (its tiny PE ops slot
        # between conv groups; part 1 frees the score/gram PSUM banks before
        # stream(b+1)'s first scores matmul needs them)
        sq_ = list(range(n_batches)) if seq is None else list(seq)
        vis = [(i, b) for i, b in enumerate(sq_)]
        n = len(vis)
        emit_load_q1(0, vis[0][1], after_c0=early_consts)
        emit_stream(0)
        late_consts()
        for i in range(1, n):
            emit_load_q1(i, vis[i][1])
            # softmax(i-1) and batch i-1's out-head hide inside stream(i)'s
            # conv groups
            sm = softmax_parts(i - 1)
            emit_stream(i, hooks=(*sm, lambda vv=i - 1: out_head(vv)))
            if i >= 2:
                emit_out(i - 2)
        if n > 1:
            # tail: last softmax chain hides under out(n-2)'s fin stream
            emit_out(n - 2, hooks=softmax_parts(n - 1))
            out_head(n - 1)
            emit_out(n - 1, split_last=True)
        else:
            emit_softmax(0)
            emit_out(0)

    if split_waits:
        _split_waits(nc)
    return nc


def _get_nc():
    if "nc" not in _CACHE:
        _CACHE["nc"] = build_nc()
    return _CACHE["nc"]


def make_inputs(inputs):
    """Host-side prep: consts + per-core sharded activations."""
    consts = _host_consts(inputs["W_kv"], inputs["W_q"], inputs["W_dw"],
                          inputs["W_po"], inputs["temperature"])
    xf = np.asarray(inputs["x"], np.float32)
    x8f = _f8(xf)
    x5f = _f8e5(xf - np.float32(x8f))
    x8 = x8f.reshape(B, 2, 128, M).transpose(0, 2, 1, 3)
    x5 = x5f.reshape(B, 2, 128, M).transpose(0, 2, 1, 3)
    qa8 = _f8(_tap_images(inputs["y"])).reshape(B, 9, 2, 128, M)
    qa8 = qa8.transpose(0, 1, 3, 2, 4)
    in_maps = []
    for i in range(NCORES):
        m = dict(consts)
        m["x8_sh"] = np.ascontiguousarray(x8[BL * i:BL * (i + 1)])
        m["x5_sh"] = np.ascontiguousarray(x5[BL * i:BL * (i + 1)])
        m["qa8_sh"] = np.ascontiguousarray(qa8[BL * i:BL * (i + 1)])
        in_maps.append(m)
    return in_maps


def run(inputs, trace=False, trace_kwargs=None):
    from concourse.bass_utils import run_bass_kernel_spmd

    nc = _get_nc()
    in_maps = make_inputs(inputs)
    res = run_bass_kernel_spmd(
        nc, in_maps, core_ids=list(range(NCORES)), trace=trace,
        trace_kwargs=trace_kwargs or {})
    out = np.concatenate(
        [np.asarray(res.results[i]["out"], np.float32) for i in range(NCORES)],
        axis=0)
    return out, res


def kernel(**inputs) -> np.ndarray:
    out, _ = run(inputs, trace=False)
    return out
